# revision 5
# baseline (speedup 1.0000x reference)
"""Trainium2 Bass kernel for DSSConv2d (low-rank spatially-combined 3x3 conv).

Computation (per reference):
  convs = conv2d(x, w.reshape(rank*oc, ic, 3, 3), pad=1)   # [B, rank*oc, H, W]
  cw    = softmax(cw_row + cw_col, axis=0)                 # [rank, H, W]
  out   = einsum('bkcxy,kxy->bcxy', convs.reshape(B,rank,oc,H,W), cw)
  out  += b + b_col + b_row

Strategy:
  - Data parallel: batch 32 -> 4 images per core on 8 cores.
  - 3x3 conv = 9 shifted 1x1 convs: for each (dy,dx), a [ic,oc] matmul over a
    shifted slice of zero-padded x, accumulated in PSUM. bf16 inputs, fp32 acc.
  - Layout: psum[oc, pix] with weights stationary; pixel tiles = 8 image rows
    (N = 8*56 = 448 <= 512 PSUM bank).
  - Per-pixel rank combine on VectorE with the softmax weights broadcast
    across partitions via a stride-0 DMA; bias plane precomputed on host.
"""

import numpy as np
import ml_dtypes
from contextlib import ExitStack

import concourse.bass as bass
import concourse.mybir as mybir
import concourse.tile as tile
from concourse import bacc
from concourse.bass_utils import run_bass_kernel_spmd

RANK, OC, IC = 4, 128, 128
B, H, W = 32, 56, 56
NCORES = 8
B_LOC = B // NCORES          # 4 images per core
HP, WP = H + 2, W + 2        # zero-padded input: 58 x 58
PIX = H * W                  # 3136
RB = 8                       # output rows per pixel tile
NBLK = H // RB               # 7 tiles per image
NT = RB * W                  # 448 pixels per tile

BF16 = mybir.dt.bfloat16
F32 = mybir.dt.float32

_CACHE = {}


def _build_nc():
    nc = bacc.Bacc()
    x_in = nc.dram_tensor("x", [B_LOC, IC, HP, WP], BF16, kind="ExternalInput")
    w_in = nc.dram_tensor("w", [IC, RANK, 9, OC], BF16, kind="ExternalInput")
    cw_in = nc.dram_tensor("cw", [RANK, PIX], F32, kind="ExternalInput")
    bias_in = nc.dram_tensor("bias", [OC, PIX], F32, kind="ExternalInput")
    out = nc.dram_tensor("out", [B_LOC, OC, PIX], F32, kind="ExternalOutput")

    with tile.TileContext(nc) as tc, ExitStack() as ctx:
        consts = ctx.enter_context(tc.tile_pool(name="consts", bufs=1))
        xpool = ctx.enter_context(tc.tile_pool(name="xpool", bufs=2))
        mpool = ctx.enter_context(tc.tile_pool(name="mpool", bufs=2))
        opool = ctx.enter_context(tc.tile_pool(name="opool", bufs=3))
        pspool = ctx.enter_context(tc.tile_pool(name="ps", bufs=2, space="PSUM"))

        w_sb = consts.tile([IC, RANK, 9, OC], BF16)
        nc.sync.dma_start(out=w_sb, in_=w_in[:, :, :, :])

        # softmax combine weights, replicated across all 128 partitions
        cwb = consts.tile([128, RANK, PIX], F32)
        nc.sync.dma_start(out=cwb, in_=cw_in[:, :].partition_broadcast(128))

        bias_sb = consts.tile([OC, PIX], F32)
        nc.sync.dma_start(out=bias_sb, in_=bias_in[:, :])

        for img in range(B_LOC):
            x_sb = xpool.tile([IC, HP, WP], BF16)
            nc.sync.dma_start(out=x_sb, in_=x_in[img])
            for blk in range(NBLK):
                h0 = blk * RB
                p0 = blk * NT
                psums = []
                for r in range(RANK):
                    ps = pspool.tile([OC, NT], F32, name=f"ps{r}", tag=f"ps{r}")
                    for o in range(9):
                        dy, dx = o // 3, o % 3
                        nc.tensor.matmul(
                            ps[:, :],
                            lhsT=w_sb[:, r, o, :],
                            rhs=x_sb[:, h0 + dy : h0 + dy + RB, dx : dx + W],
                            start=(o == 0),
                            stop=(o == 8),
                        )
                    psums.append(ps)
                ms = []
                for r in range(RANK):
                    m = mpool.tile([OC, NT], F32, name=f"m{r}", tag=f"m{r}")
                    nc.vector.tensor_mul(m, psums[r], cwb[:, r, p0 : p0 + NT])
                    ms.append(m)
                s0 = mpool.tile([OC, NT], F32, name="s0", tag="s0")
                nc.vector.tensor_add(s0, ms[0], ms[1])
                s1 = mpool.tile([OC, NT], F32, name="s1", tag="s1")
                nc.vector.tensor_add(s1, ms[2], ms[3])
                s2 = mpool.tile([OC, NT], F32, name="s2", tag="s2")
                nc.vector.tensor_add(s2, s0, s1)
                o_t = opool.tile([OC, NT], F32, name="ot", tag="ot")
                nc.vector.tensor_add(o_t, s2, bias_sb[:, p0 : p0 + NT])
                nc.sync.dma_start(
                    out=out[img][:, p0 : p0 + NT], in_=o_t
                )
    nc.finalize()
    return nc


def _prep_inputs(x, w, cw_row, cw_col, b_row, b_col, b):
    # zero-padded bf16 input
    xp = np.zeros((B, IC, HP, WP), dtype=ml_dtypes.bfloat16)
    xp[:, :, 1 : H + 1, 1 : W + 1] = x.astype(ml_dtypes.bfloat16)

    # weights: [rank, oc, ic, kh, kw] -> [ic, rank, kh*kw, oc], bf16
    wt = np.ascontiguousarray(np.transpose(w, (2, 0, 3, 4, 1))).reshape(
        IC, RANK, 9, OC
    ).astype(ml_dtypes.bfloat16)

    # softmax over rank of per-pixel combine logits
    logits = (cw_row + cw_col).astype(np.float64)  # [rank, H, W]
    logits -= logits.max(axis=0, keepdims=True)
    e = np.exp(logits)
    cw = (e / e.sum(axis=0, keepdims=True)).astype(np.float32).reshape(RANK, PIX)

    # combined bias plane [oc, pix]
    bias = (
        b.reshape(OC, 1, 1) + b_row.reshape(1, H, 1) + b_col.reshape(1, 1, W)
    ).astype(np.float32).reshape(OC, PIX)

    return xp, wt, cw, bias


def _run(inputs, trace=False):
    if "nc" not in _CACHE:
        _CACHE["nc"] = _build_nc()
    nc = _CACHE["nc"]
    xp, wt, cw, bias = _prep_inputs(**inputs)
    in_maps = [
        {"x": xp[c * B_LOC : (c + 1) * B_LOC], "w": wt, "cw": cw, "bias": bias}
        for c in range(NCORES)
    ]
    res = run_bass_kernel_spmd(nc, in_maps, list(range(NCORES)), trace=trace)
    outs = [np.asarray(res.results[c]["out"]) for c in range(NCORES)]
    full = np.concatenate(outs, axis=0).reshape(B, OC, H, W).astype(np.float32)
    return full, res


def kernel(**inputs):
    full, _ = _run(inputs)
    return full


# revision 8
# speedup vs baseline: 1.1296x; 1.1296x over previous
"""Trainium2 Bass kernel for DSSConv2d (low-rank spatially-combined 3x3 conv).

Computation (per reference):
  convs = conv2d(x, w.reshape(rank*oc, ic, 3, 3), pad=1)   # [B, rank*oc, H, W]
  cw    = softmax(cw_row + cw_col, axis=0)                 # [rank, H, W]
  out   = einsum('bkcxy,kxy->bcxy', convs.reshape(B,rank,oc,H,W), cw)
  out  += b + b_col + b_row

Strategy:
  - Data parallel: batch 32 -> 4 images per core on 8 cores.
  - 3x3 conv = 9 shifted 1x1 convs: for each (dy,dx), a [ic,oc] matmul over a
    shifted slice of zero-padded x, accumulated in PSUM. bf16 inputs, fp32 acc.
  - Layout: psum[oc, pix] with weights stationary; pixel tiles = 8 image rows
    (N = 8*56 = 448 <= 512 PSUM bank).
  - Per-pixel rank combine on VectorE with the softmax weights broadcast
    across partitions via a stride-0 DMA; bias plane precomputed on host.
"""

import numpy as np
import ml_dtypes
from contextlib import ExitStack

import concourse.bass as bass
import concourse.mybir as mybir
import concourse.tile as tile
from concourse import bacc
from concourse.bass_utils import run_bass_kernel_spmd

RANK, OC, IC = 4, 128, 128
B, H, W = 32, 56, 56
NCORES = 8
B_LOC = B // NCORES          # 4 images per core
HP, WP = H + 2, W + 2        # zero-padded input: 58 x 58
PIX = H * W                  # 3136
RB = 8                       # output rows per pixel tile
NBLK = H // RB               # 7 tiles per image
NT = RB * W                  # 448 pixels per tile

BF16 = mybir.dt.bfloat16
F32 = mybir.dt.float32

_CACHE = {}


def _build_nc():
    nc = bacc.Bacc()
    x_in = nc.dram_tensor("x", [B_LOC, IC, HP, WP], BF16, kind="ExternalInput")
    w_in = nc.dram_tensor("w", [IC, RANK, 9, OC], BF16, kind="ExternalInput")
    cw_in = nc.dram_tensor("cw", [RANK, PIX], BF16, kind="ExternalInput")
    bias_in = nc.dram_tensor("bias", [OC, PIX], F32, kind="ExternalInput")
    out = nc.dram_tensor("out", [B_LOC, OC, PIX], F32, kind="ExternalOutput")

    with tile.TileContext(nc) as tc, ExitStack() as ctx:
        consts = ctx.enter_context(tc.tile_pool(name="consts", bufs=1))
        xpool = ctx.enter_context(tc.tile_pool(name="xpool", bufs=2))
        mpool = ctx.enter_context(tc.tile_pool(name="mpool", bufs=2))
        opool = ctx.enter_context(tc.tile_pool(name="opool", bufs=3))
        pspool = ctx.enter_context(tc.tile_pool(name="ps", bufs=2, space="PSUM"))

        # DMA priority order: weights first (PE needs them immediately),
        # then the first image rows, then combine weights / bias.
        w_sb = consts.tile([IC, RANK, 9, OC], BF16)
        nc.sync.dma_start(out=w_sb, in_=w_in[:, :, :, :])

        # x split into top rows [0, 34) and bottom rows [32, 58) so the PE
        # can start after ~0.5 MB instead of a full image.
        HT, HB0 = 34, 32  # top tile rows, bottom tile start row
        HBN = HP - HB0    # 26

        def load_img(img):
            xt = xpool.tile([IC, HT, WP], BF16, name="xt", tag="xt")
            nc.sync.dma_start(out=xt, in_=x_in[img][:, 0:HT, :])
            xb = xpool.tile([IC, HBN, WP], BF16, name="xb", tag="xb")
            nc.sync.dma_start(out=xb, in_=x_in[img][:, HB0:HP, :])
            return xt, xb

        x0 = load_img(0)

        # softmax combine weights (bf16), replicated across all 128 partitions
        cwb = consts.tile([128, RANK, PIX], BF16)
        nc.sync.dma_start(out=cwb, in_=cw_in[:, :].partition_broadcast(128))

        bias_sb = consts.tile([OC, PIX], F32)
        nc.sync.dma_start(out=bias_sb, in_=bias_in[:, :])

        x_cur = x0
        for img in range(B_LOC):
            xt, xb = x_cur
            if img + 1 < B_LOC:
                x_nxt = load_img(img + 1)
            for blk in range(NBLK):
                h0 = blk * RB
                p0 = blk * NT
                psums = []
                for r in range(RANK):
                    ps = pspool.tile([OC, NT], F32, name=f"ps{r}", tag=f"ps{r}")
                    for o in range(9):
                        dy, dx = o // 3, o % 3
                        if h0 + dy + RB <= HT:
                            rhs = xt[:, h0 + dy : h0 + dy + RB, dx : dx + W]
                        else:
                            hb = h0 - HB0
                            rhs = xb[:, hb + dy : hb + dy + RB, dx : dx + W]
                        nc.tensor.matmul(
                            ps[:, :],
                            lhsT=w_sb[:, r, o, :],
                            rhs=rhs,
                            start=(o == 0),
                            stop=(o == 8),
                        )
                    psums.append(ps)
                # combine: chained so the DVE tail after the last matmul is
                # only (mul3, add, add-bias)
                acc = None
                for r in range(RANK):
                    m = mpool.tile([OC, NT], F32, name=f"m{r}", tag=f"m{r}")
                    nc.vector.tensor_mul(m, psums[r], cwb[:, r, p0 : p0 + NT])
                    if acc is None:
                        acc = m
                    else:
                        a = mpool.tile([OC, NT], F32, name=f"a{r}", tag=f"a{r}")
                        nc.vector.tensor_add(a, acc, m)
                        acc = a
                o_t = opool.tile([OC, NT], F32, name="ot", tag="ot")
                nc.vector.tensor_add(o_t, acc, bias_sb[:, p0 : p0 + NT])
                nc.sync.dma_start(
                    out=out[img][:, p0 : p0 + NT], in_=o_t
                )
            if img + 1 < B_LOC:
                x_cur = x_nxt
    nc.finalize()
    return nc


def _prep_inputs(x, w, cw_row, cw_col, b_row, b_col, b):
    # zero-padded bf16 input
    xp = np.zeros((B, IC, HP, WP), dtype=ml_dtypes.bfloat16)
    xp[:, :, 1 : H + 1, 1 : W + 1] = x.astype(ml_dtypes.bfloat16)

    # weights: [rank, oc, ic, kh, kw] -> [ic, rank, kh*kw, oc], bf16
    wt = np.ascontiguousarray(np.transpose(w, (2, 0, 3, 4, 1))).reshape(
        IC, RANK, 9, OC
    ).astype(ml_dtypes.bfloat16)

    # softmax over rank of per-pixel combine logits
    logits = (cw_row + cw_col).astype(np.float64)  # [rank, H, W]
    logits -= logits.max(axis=0, keepdims=True)
    e = np.exp(logits)
    cw = (
        (e / e.sum(axis=0, keepdims=True))
        .astype(ml_dtypes.bfloat16)
        .reshape(RANK, PIX)
    )

    # combined bias plane [oc, pix]
    bias = (
        b.reshape(OC, 1, 1) + b_row.reshape(1, H, 1) + b_col.reshape(1, 1, W)
    ).astype(np.float32).reshape(OC, PIX)

    return xp, wt, cw, bias


def _run(inputs, trace=False):
    if "nc" not in _CACHE:
        _CACHE["nc"] = _build_nc()
    nc = _CACHE["nc"]
    xp, wt, cw, bias = _prep_inputs(**inputs)
    in_maps = [
        {"x": xp[c * B_LOC : (c + 1) * B_LOC], "w": wt, "cw": cw, "bias": bias}
        for c in range(NCORES)
    ]
    res = run_bass_kernel_spmd(nc, in_maps, list(range(NCORES)), trace=trace)
    outs = [np.asarray(res.results[c]["out"]) for c in range(NCORES)]
    full = np.concatenate(outs, axis=0).reshape(B, OC, H, W).astype(np.float32)
    return full, res


def kernel(**inputs):
    full, _ = _run(inputs)
    return full
